# revision 11
# baseline (speedup 1.0000x reference)
"""Trainium2 Bass kernel for nn_Dependency_GATLayer (gnn_message_passing).

Problem structure (N=8192 nodes, D=256, E=N-1 edges):
  Hx = x @ W.T
  s_e = [Hx[gov_e]; Hx[dep_e]] @ a          (per-edge logit)
  e_tensor[gov_e, dep_e] = s_e, masked row-softmax on governor rows
  h[dep_e] = Hx[gov_e]; h[gov_e] += attn[gov_e, dep_e] * Hx[dep_e]
  out = leaky_relu(h, 0.2)

Key simplifications used (and verified at runtime):
  * dep == arange(1, N): h-base is a pure row gather of Hx by gov.
  * each governor appears at most once in gov => every governor row of
    e_tensor has exactly ONE nonzero entry, so the masked softmax
    collapses to: coef_e = 1.0 if s_e > 0 else 1/N.

All gathers use indices known at kernel() call time, so the host
pre-permutes ROWS OF THE INPUT x (pure data staging; x@W.T commutes
with row permutation) and the device does only matmuls + elementwise:
  A[i]   = xg[i] @ W.T     with xg[i] = x[gov[i-1]]          (h base)
  B[i]   = xp2[i] @ W.T    with xp2[i] = x[invgov[i]+1]      (scatter term)
  s[i]   = x[i]@(W.T a_g) + xp2[i]@(W.T a_d)                 (exact fp32)
  coef   = m2 * (s>0 ? 1 : 1/N)
  out[i] = leaky_relu(A[i] + coef[i]*B[i], 0.2)

Sharding: nodes (rows) split evenly across the 8 cores; W/a replicated;
no collectives. Everything on-device runs in transposed layout
[feature, node] so DMA is contiguous and matmuls contract on partitions.
"""

import sys
import types

import numpy as np

N = 8192
D = 256
NCORES = 8
NPC = N // NCORES  # nodes per core = 1024
FCH = 512          # free-dim chunk (one PSUM bank of fp32)
ALPHA = 0.2

# A/B matmul precision: "f32r" (full-rate, ~19-bit), "bf16", or "f32" (4x slower)
MM_DTYPE = "f32r"
_COMPILED = {}


def _install_ntff_hook_shim():
    """Allow run_bass_kernel_spmd(trace=True) under axon: provide the
    antenv.axon_hooks module the image lacks, backed by the ctypes NTFF
    driver from trn_agent_boot."""
    if "antenv.axon_hooks" in sys.modules:
        return
    try:
        from trn_agent_boot.trn_boot import _ntff_profile_via_ctypes
        hook = _ntff_profile_via_ctypes("/opt/axon/libaxon_pjrt.so")
    except Exception:
        hook = None
    mod = types.ModuleType("antenv.axon_hooks")
    mod.get_axon_ntff_profile_hook = lambda: hook
    mod.set_axon_ntff_profile_hook = lambda h: None
    sys.modules["antenv.axon_hooks"] = mod


def _build_program():
    """Build the SPMD Bass program (same for every core)."""
    import concourse.bass as bass
    import concourse.tile as tile
    from concourse import mybir
    from concourse.vector_clock import ScopedClock

    import bass_rust

    MAXW = 1  # this walrus build allows only one sync wait per instruction

    class _TC(tile.TileContext):
        def schedule_and_allocate(self):
            ret = super().schedule_and_allocate()
            # Hoist excess sync waits onto same-engine nops (in-order
            # execution makes a preceding nop-with-wait equivalent).
            for bb in self.nc.m.functions[0].blocks:
                insts = bb.instructions
                out = []
                changed = False
                for inst in insts:
                    si = inst.sync_info
                    waits = list(si.on_wait) if si else []
                    maxw = MAXW
                    if len(waits) > maxw:
                        changed = True
                        extra = waits[: len(waits) - maxw]
                        keep = waits[len(waits) - maxw :]
                        for j in range(0, len(extra), MAXW):
                            nop = mybir.InstNoOp(
                                name=self.nc.get_next_instruction_name(),
                                ins=[],
                                outs=[],
                            )
                            nop.engine = inst.engine
                            nop.sync_info = bass_rust.SyncInfo(
                                on_wait=extra[j : j + MAXW], on_update=[]
                            )
                            out.append(nop)
                        inst.sync_info = bass_rust.SyncInfo(
                            on_wait=keep, on_update=list(si.on_update)
                        )
                    out.append(inst)
                if changed:
                    bb.instructions = out
            return ret

        # walrus CTRL codegen rejects >2 sync waits on one instruction;
        # split the tail-drain waits into single-wait instructions.
        def _drain_and_barrier(self, tick_clock, wait_clock):
            probe = mybir.InstNoOp(
                name=self.nc.get_next_instruction_name(), ins=[], outs=[]
            )
            probe.engine = mybir.EngineType.SP
            wait_clock.add_sem_waits(
                probe, ScopedClock({None: tick_clock.global_clock})
            )
            waits = list(probe.sync_info.on_wait) if probe.sync_info else []
            assert self.sems is not None
            sem_by_name = {h.name: h for h in self.sems.allocated().values()}
            for w in waits:
                self.nc.sync.wait_ge(sem_by_name[w.ant_name], w.wait_value)
            self.nc.sync.drain()
            self.nc.all_engine_barrier()
            popped = self.nc._tile_sem_poison_stack.pop()
            assert popped is self._sem_poison
            self.nc.clear_and_free_semaphores(list(self.sems.allocated().values()))
            self.nc.all_engine_barrier()

    dt = mybir.dt
    f32 = dt.float32
    if MM_DTYPE == "bf16":
        mmdt = dt.bfloat16
    elif MM_DTYPE == "f32r":
        mmdt = dt.float32r
    else:
        mmdt = dt.float32
    # dtype of the DMAed xg / W tensors (bf16 path ships half-size tensors)
    io_mmdt = dt.bfloat16 if MM_DTYPE == "bf16" else f32

    nc = bass.Bass()
    xT_d = nc.declare_dram_parameter("xT", [D, NPC], f32, isOutput=False)
    xgT_d = nc.declare_dram_parameter("xgT", [D, NPC], mmdt, isOutput=False)
    xp2T_d = nc.declare_dram_parameter("xp2T", [D, NPC], f32, isOutput=False)
    wt_d = nc.declare_dram_parameter("wt", [D, D], mmdt, isOutput=False)
    wgwd_d = nc.declare_dram_parameter("wgwd", [128, 4], f32, isOutput=False)
    m2a_d = nc.declare_dram_parameter("m2a", [1, NPC], f32, isOutput=False)
    m2b_d = nc.declare_dram_parameter("m2b", [1, NPC], f32, isOutput=False)
    ones_d = nc.declare_dram_parameter("ones", [1, 128], mmdt, isOutput=False)
    out_d = nc.declare_dram_parameter("outT", [D, NPC], f32, isOutput=True)

    KCH = D // 128  # 2 contraction chunks
    NF = NPC // FCH  # 2 free chunks
    Alu = mybir.AluOpType

    def mm(ap):
        return ap

    with _TC(nc) as tc:
        with (
            tc.tile_pool(name="const", bufs=1) as cpool,
            tc.tile_pool(name="xin", bufs=1) as xpool,
            tc.tile_pool(name="work", bufs=1) as wpool,
            tc.tile_pool(name="coef", bufs=2) as coefpool,
            tc.tile_pool(name="out", bufs=1) as opool,
            tc.tile_pool(name="ps_h", bufs=4, space="PSUM") as ps_h_pool,
            tc.tile_pool(name="ps_s", bufs=2, space="PSUM") as ps_s_pool,
            tc.tile_pool(name="ps_b", bufs=2, space="PSUM") as ps_b_pool,
        ):
            # --- constants ---
            wt_sb = [cpool.tile([128, D], mmdt, tag=f"wt{k}", name=f"wt{k}") for k in range(KCH)]
            for k in range(KCH):
                nc.sync.dma_start(wt_sb[k][:], wt_d[128 * k : 128 * (k + 1), :])
            wgwd_sb = cpool.tile([128, 4], f32, tag="wgwd", name="wgwd")
            nc.sync.dma_start(wgwd_sb[:], wgwd_d[:])
            m2a_sb = cpool.tile([1, NPC], f32, tag="m2a", name="m2a")
            nc.sync.dma_start(m2a_sb[:], m2a_d[:])
            m2b_sb = cpool.tile([1, NPC], f32, tag="m2b", name="m2b")
            nc.sync.dma_start(m2b_sb[:], m2b_d[:])
            ones_sb = cpool.tile([1, 128], mmdt, tag="ones", name="ones")
            nc.sync.dma_start(ones_sb[:], ones_d[:])

            # --- inputs (full per-core width; sliced per f-chunk below) ---
            xT_sb = [xpool.tile([128, NPC], f32, tag=f"xT{k}", name=f"xT{k}") for k in range(KCH)]
            xgT_sb = [xpool.tile([128, NPC], mmdt, tag=f"xgT{k}", name=f"xgT{k}") for k in range(KCH)]
            xp2T_sb = [xpool.tile([128, NPC], f32, tag=f"xp2T{k}", name=f"xp2T{k}") for k in range(KCH)]
            for k in range(KCH):
                sl = slice(128 * k, 128 * (k + 1))
                nc.sync.dma_start(xT_sb[k][:], xT_d[sl, :])
                nc.sync.dma_start(xgT_sb[k][:], xgT_d[sl, :])
                nc.sync.dma_start(xp2T_sb[k][:], xp2T_d[sl, :])

            out_sb = [opool.tile([128, NPC], f32, tag=f"out{d}", name=f"outsb{d}") for d in range(KCH)]

            for f in range(NF):
                fs = slice(FCH * f, FCH * (f + 1))
                # --- s = x@wg + xp2@wd  (exact fp32 matvec on PE) ---
                ps_s = ps_s_pool.tile([1, FCH], f32, tag="s", name=f"ps_s{f}")
                nc.tensor.matmul(ps_s[:], wgwd_sb[:, 0:1], xT_sb[0][:, fs], start=True, stop=False)
                nc.tensor.matmul(ps_s[:], wgwd_sb[:, 1:2], xT_sb[1][:, fs], start=False, stop=False)
                nc.tensor.matmul(ps_s[:], wgwd_sb[:, 2:3], xp2T_sb[0][:, fs], start=False, stop=False)
                nc.tensor.matmul(ps_s[:], wgwd_sb[:, 3:4], xp2T_sb[1][:, fs], start=False, stop=True)

                # --- coef = (s>0)*m2a + m2b  in {0, 1, 1/N} ---
                coef_sb = coefpool.tile([1, FCH], f32, tag="coef", name=f"coef{f}")
                nc.vector.scalar_tensor_tensor(
                    coef_sb[:], ps_s[:], 0.0, m2a_sb[:, fs], Alu.is_gt, Alu.mult
                )
                coef_mm = coefpool.tile([1, FCH], mmdt, tag="coefmm", name=f"coefmm{f}")
                nc.vector.tensor_tensor(coef_mm[:], coef_sb[:], m2b_sb[:, fs], Alu.add)

                # --- broadcast coef across partitions via K=1 matmul ---
                ps_b = ps_b_pool.tile([128, FCH], f32, tag="bc", name=f"ps_b{f}")
                nc.tensor.matmul(ps_b[:], mm(ones_sb[:]), mm(coef_mm[:]), start=True, stop=True)

                # --- xp2s = coef * xp2 (feeds B matmul) ---
                xp2s_sb = [
                    wpool.tile([128, FCH], mmdt, tag=f"xp2s{k}{f}", name=f"xp2s{k}_{f}") for k in range(KCH)
                ]
                for k in range(KCH):
                    nc.vector.tensor_tensor(
                        xp2s_sb[k][:], xp2T_sb[k][:, fs], ps_b[:], Alu.mult
                    )

                # --- h = xg@W.T + xp2s@W.T  (PSUM-accumulated), leaky, out ---
                for dch in range(KCH):
                    ds = slice(128 * dch, 128 * (dch + 1))
                    ps = ps_h_pool.tile([128, FCH], f32, tag="h", name=f"ps_h{dch}_{f}")
                    nc.tensor.matmul(ps[:], mm(wt_sb[0][:, ds]), mm(xgT_sb[0][:, fs]), start=True, stop=False)
                    nc.tensor.matmul(ps[:], mm(wt_sb[1][:, ds]), mm(xgT_sb[1][:, fs]), start=False, stop=False)
                    nc.tensor.matmul(ps[:], mm(wt_sb[0][:, ds]), mm(xp2s_sb[0][:]), start=False, stop=False)
                    nc.tensor.matmul(ps[:], mm(wt_sb[1][:, ds]), mm(xp2s_sb[1][:]), start=False, stop=True)
                    # leaky_relu: out = max(0.2*h, h). DVE may read PSUM only
                    # once per op, so stage h in SBUF first.
                    h_sb = wpool.tile([128, FCH], f32, tag=f"h{dch}{f}", name=f"h{dch}_{f}")
                    nc.vector.tensor_copy(h_sb[:], ps[:])
                    nc.vector.scalar_tensor_tensor(
                        out_sb[dch][:, fs], h_sb[:], ALPHA, h_sb[:], Alu.mult, Alu.max
                    )

            for dch in range(KCH):
                nc.sync.dma_start(out_d[128 * dch : 128 * (dch + 1), :], out_sb[dch][:])

    return nc


def _get_program():
    key = MM_DTYPE
    if key not in _COMPILED:
        _COMPILED[key] = _build_program()
    return _COMPILED[key]


def _prep_inputs(x, W, a, dep, gov):
    """Host-side sharding/staging: row permutations of x, weight folding."""
    import ml_dtypes

    x = np.asarray(x, np.float32)
    W = np.asarray(W, np.float32)
    a = np.asarray(a, np.float32)
    dep = np.asarray(dep)
    gov = np.asarray(gov)
    n, d = x.shape

    # weight folding (W, a are weights; indices only otherwise)
    Wt = np.ascontiguousarray(W.T)  # [k, d]
    wg = (W.T.astype(np.float64) @ a[:d].astype(np.float64)).astype(np.float32)
    wd = (W.T.astype(np.float64) @ a[d:].astype(np.float64)).astype(np.float32)
    wgwd = np.ascontiguousarray(
        np.stack([wg[:128], wg[128:], wd[:128], wd[128:]], axis=1)
    )  # [128, 4]

    # index plumbing
    invgov = np.full(n, -1, np.int64)
    invgov[gov] = np.arange(len(gov))
    m2 = (invgov >= 0).astype(np.float32)

    xg = np.zeros_like(x)
    xg[dep] = x[gov]  # dep is a permutation of 1..n-1
    xp2 = np.zeros_like(x)
    sel = invgov >= 0
    xp2[sel] = x[invgov[sel] + 1]

    m2a = m2 * np.float32(1.0 - 1.0 / n)
    m2b = m2 * np.float32(1.0 / n)

    io_np = ml_dtypes.bfloat16 if MM_DTYPE == "bf16" else np.float32
    wt_io = np.ascontiguousarray(Wt.astype(io_np))
    ones_io = np.ones((1, 128), io_np)

    xT = x.T
    xgT = xg.T.astype(io_np)
    xp2T = xp2.T

    in_maps = []
    for c in range(NCORES):
        sl = slice(NPC * c, NPC * (c + 1))
        in_maps.append(
            {
                "xT": np.ascontiguousarray(xT[:, sl]),
                "xgT": np.ascontiguousarray(xgT[:, sl]),
                "xp2T": np.ascontiguousarray(xp2T[:, sl]),
                "wt": wt_io,
                "wgwd": wgwd,
                "m2a": np.ascontiguousarray(m2a[sl][None, :]),
                "m2b": np.ascontiguousarray(m2b[sl][None, :]),
                "ones": ones_io,
            }
        )
    return in_maps


def _fallback_numpy(x, W, a, dep, gov):
    """Reference-exact general path (duplicate governors); CPU only."""
    x = np.asarray(x, np.float64)
    W = np.asarray(W, np.float64)
    a = np.asarray(a, np.float64)
    n, d = x.shape
    Hx = x @ W.T
    s = np.concatenate([Hx[gov], Hx[dep]], axis=-1) @ a
    e = np.zeros((n, n))
    e[gov, dep] = s
    gov_mask = np.zeros(n, bool)
    gov_mask[gov] = True
    masked = np.where(e > 0, e, -1e18)
    mx = masked.max(axis=1, keepdims=True)
    ex = np.exp(masked - mx)
    sm = ex / ex.sum(axis=1, keepdims=True)
    attn = np.where(gov_mask[:, None], sm, e)
    h = np.zeros((n, d))
    h[dep] = Hx[gov]
    coef = attn[gov, dep]
    np.add.at(h, gov, coef[:, None] * Hx[dep])
    return np.where(h > 0, h, ALPHA * h).astype(np.float32)


def kernel(x, W, a, dep, gov, _trace=False, _tmpdir=None):
    x = np.asarray(x)
    W = np.asarray(W)
    a = np.asarray(a)
    dep = np.asarray(dep)
    gov = np.asarray(gov)

    # Assumptions baked into the device program; fall back if violated.
    ok = (
        x.shape == (N, D)
        and dep.shape == (N - 1,)
        and np.array_equal(dep, np.arange(1, N, dtype=dep.dtype))
        and len(np.unique(gov)) == len(gov)
    )
    if not ok:
        return _fallback_numpy(x, W, a, dep, gov)

    _install_ntff_hook_shim()
    import concourse.bass_utils as bass_utils

    bass_utils.upload_artifacts = lambda tmpdir: f"local:{tmpdir}"

    nc = _get_program()
    in_maps = _prep_inputs(x, W, a, dep, gov)
    res = bass_utils.run_bass_kernel_spmd(
        nc,
        in_maps,
        core_ids=list(range(NCORES)),
        trace=_trace,
        tmpdir=_tmpdir,
    )
    out = np.empty((N, D), np.float32)
    for c in range(NCORES):
        out[NPC * c : NPC * (c + 1), :] = res.results[c]["outT"].T
    if _trace:
        kernel.last_exec_time_ns = res.exec_time_ns
        kernel.last_results = res
    return out
